# revision 3
# baseline (speedup 1.0000x reference)
"""Trainium2 Bass kernel for nn_Decoder_11355893531219 (geo-biased cross-attention decoder).

Sharding: 8 cores = 4 batches x 2 query-halves. Each core computes its
[batch, 512-query-slice] of both branches; an AllGather between core pairs
exchanges the updated `aas` (K-source of the struct branch).

Layout: activations live TRANSPOSED in SBUF ([feature, token]) so every matmul
contracts over the partition dim. Attention computes logits^T [keys, m] per
head; softmax denominator via a ones-column matmul; normalization via a
PE-broadcast of the reciprocal row. The RBF bias is rebuilt per (branch, head,
key-tile) from a shared d2 tile with a fused Relu / max (lower clip; the upper
clip at 2*log(0.99) and global shifts are dropped - softmax is shift-invariant
and the upper clamp is active only for pairs closer than ~0.85 A, a ~2e-2
logit perturbation on a vanishing fraction of entries). Matmuls run in
float32r (full PE speed, ~1.5e-4 rel err).
"""
import numpy as np

import concourse.bass as bass
import concourse.bacc as bacc
import concourse.tile as tile
from concourse import mybir
from concourse import bass2jax

F32 = mybir.dt.float32
F32R = mybir.dt.float32r
BF16 = mybir.dt.bfloat16
AF = mybir.ActivationFunctionType
OP = mybir.AluOpType

P = 128
B, N, D, H, DH, DFF = 4, 1024, 512, 8, 64, 1024
M = 512               # query rows per core
KT = N // P           # 8 key tiles
DT = D // P           # 4 feature tiles
FT = DFF // P         # 8 hidden tiles
BETA = 2.0
LN_EPS = 1e-5
LO = BETA * float(np.log(0.01))      # -9.2103404
SPREADS = 1.0 * (6.0 / 1.0) ** (np.arange(H) / (H - 1))
CH = (-BETA / (2.0 * SPREADS ** 2)).astype(np.float64)   # per-head scale on d2
PAD_NEG = -30000.0


def _din(nc, name, shape, dt):
    return nc.dram_tensor(name, shape, dt, kind="ExternalInput").ap()


def build(nc):
    # ---- DRAM I/O ----
    aasT_s = _din(nc, "aasT_s", [D, M], F32R)
    wfT_d = _din(nc, "wfT", [D, N], F32R)
    wfT_m_d = _din(nc, "wfT_m", [D, M], F32R)
    coordsT_d = _din(nc, "coordsT", [3, N], F32)
    coordsT_m_d = _din(nc, "coordsT_m", [3, M], F32)
    coords_n_d = _din(nc, "coords_n", [N, 3], F32)
    pad_d = _din(nc, "pad", [P, KT], F32)
    ident_d = _din(nc, "ident", [P, P], F32R)
    ones_col_d = _din(nc, "ones_col", [P, 1], F32R)
    ones_row_d = _din(nc, "ones_row", [1, P], F32R)
    ones3_d = _din(nc, "ones3", [3, 1], F32)
    ones_row_f_d = _din(nc, "ones_row_f", [1, P], F32)
    Wd = {}
    for pfx in ("seq", "str"):
        for wn, shp in (("wq", [D, D]), ("wk", [D, D]), ("wv", [D, D]),
                        ("wo", [D, D]), ("w1", [D, DFF]), ("w2", [DFF, D])):
            Wd[pfx + wn] = _din(nc, f"{pfx}_{wn}", shp, F32R)
    aas_out = nc.dram_tensor("aas_outT", [D, M], F32R, kind="ExternalOutput").ap()
    wf_out = nc.dram_tensor("wf_outT", [D, M], F32R, kind="ExternalOutput").ap()

    with tile.TileContext(nc) as tc:
        with (
            tc.tile_pool(name="pb1", bufs=1) as pb1,
            tc.tile_pool(name="pb2", bufs=2) as pb2,
            tc.tile_pool(name="pb3", bufs=3) as pb3,
            tc.tile_pool(name="pln", bufs=4) as pln,
            tc.tile_pool(name="prot", bufs=2, space="PSUM") as prot,
            tc.tile_pool(name="prow", bufs=1, space="PSUM") as prow,
            tc.tile_pool(name="pacc", bufs=1, space="PSUM") as pacc,
            tc.tile_pool(name="pacc4", bufs=4, space="PSUM") as pacc4,
            tc.tile_pool(name="dram", bufs=1, space="DRAM") as dram,
        ):
            # ---- consts & global inputs ----
            ident = pb1.tile([P, P], F32R, tag="ident")
            nc.sync.dma_start(out=ident, in_=ident_d)
            ones_col = pb1.tile([P, 1], F32R, tag="ones_col")
            nc.sync.dma_start(out=ones_col, in_=ones_col_d)
            ones_row = pb1.tile([1, P], F32R, tag="ones_row")
            nc.sync.dma_start(out=ones_row, in_=ones_row_d)
            ones3 = pb1.tile([3, 1], F32, tag="ones3")
            nc.sync.dma_start(out=ones3, in_=ones3_d)
            ones_row_f = pb1.tile([1, P], F32, tag="ones_row_f")
            nc.sync.dma_start(out=ones_row_f, in_=ones_row_f_d)
            pad_sb = pb1.tile([P, KT], F32, tag="pad")
            nc.sync.dma_start(out=pad_sb, in_=pad_d)
            neglo_t = pb1.tile([P, 1], F32, tag="neglo")
            nc.vector.memset(neglo_t, -LO)
            eps_t = pb1.tile([1, 1], F32, tag="eps")
            nc.vector.memset(eps_t, LN_EPS)

            wfT = pb1.tile([P, DT, N], F32R, tag="wfT")
            nc.sync.dma_start(out=wfT, in_=wfT_d.rearrange("(k p) n -> p k n", p=P))
            aasT = pb1.tile([P, DT, M], F32R, tag="aasT")
            nc.sync.dma_start(out=aasT, in_=aasT_s.rearrange("(k p) m -> p k m", p=P))
            wfTm = pb1.tile([P, DT, M], F32R, tag="wfTm")
            nc.sync.dma_start(out=wfTm, in_=wfT_m_d.rearrange("(k p) m -> p k m", p=P))

            # ---- d2 (squared pairwise distances, [keys, m], bf16) ----
            c3k = pb2.tile([3, N], F32, tag="ktile")      # shares the kT slot (dead before attn)
            nc.sync.dma_start(out=c3k, in_=coordsT_d)
            c3m = pb1.tile([3, M], F32, tag="c3m")
            nc.sync.dma_start(out=c3m, in_=coordsT_m_d)
            cn = pb1.tile([P, KT, 3], F32, tag="cn")
            nc.sync.dma_start(out=cn, in_=coords_n_d.rearrange("(kt p) c -> p kt c", p=P))

            cn2 = pb1.tile([P, KT, 3], F32, tag="cn2")
            nc.vector.tensor_mul(out=cn2, in0=cn, in1=cn)
            nck = pb1.tile([P, KT], F32, tag="nck")
            nc.vector.reduce_sum(out=nck, in_=cn2, axis=mybir.AxisListType.X)

            sq3 = pb1.tile([3, M], F32, tag="sq3")
            nc.vector.tensor_mul(out=sq3, in0=c3m, in1=c3m)
            ncq_ps = prow.tile([1, M], F32, tag="row")
            nc.tensor.matmul(ncq_ps[0:1, :], ones3[:, 0:1], sq3, start=True, stop=True)
            ncq_row = pb1.tile([1, M], F32, tag="ncq_row")
            nc.scalar.activation(out=ncq_row, in_=ncq_ps[0:1, :], func=AF.Copy)
            ncqbc_ps = prot.tile([P, M], F32, tag="rot")
            nc.tensor.matmul(ncqbc_ps[:], ones_row_f[0:1, :], ncq_row, start=True, stop=True)
            ncqbc = pb2.tile([P, M], F32, tag="bs")       # shares the bs slot
            nc.scalar.activation(out=ncqbc, in_=ncqbc_ps, func=AF.Copy)

            d2 = pb1.tile([P, KT, M], BF16, tag="d2")
            for kt in range(KT):
                cps = prot.tile([P, M], F32, tag="rot")
                nc.tensor.matmul(cps[:], c3k[:, kt * P:(kt + 1) * P], c3m, start=True, stop=True)
                t1 = pb2.tile([P, M], F32, tag="d2t")
                nc.vector.tensor_scalar(out=t1, in0=cps, scalar1=-2.0,
                                        scalar2=nck[:, kt:kt + 1], op0=OP.mult, op1=OP.add)
                nc.vector.tensor_tensor(out=d2[:, kt, :], in0=t1, in1=ncqbc, op=OP.add)

            def emit_branch(pfx, qx, ksrc_fn, resid, out_d):
                """qx/resid: [P, DT, M] f32r tiles. ksrc_fn(kt)->AP [P, 2, M] f32r.
                Returns final [P, DT, M] f32r tile (also DMAed to out_d)."""
                # qT [P, DT, M] : head-pair-major transposed queries, pre-scaled 1/8
                wq = pb2.tile([P, DT, D], F32R, tag="w8", name=f"{pfx}_wq_sb")
                nc.sync.dma_start(out=wq, in_=Wd[pfx + "wq"].rearrange("(k p) o -> p k o", p=P))
                qt = pb1.tile([P, DT, M], F32R, tag="qt", name=f"{pfx}_qt")
                for mt in range(DT):
                    ps = prot.tile([P, M], F32, tag="rot")
                    for kt in range(DT):
                        nc.tensor.matmul(ps[:], wq[:, kt, mt * P:(mt + 1) * P], qx[:, kt, :],
                                         start=(kt == 0), stop=(kt == DT - 1))
                    nc.scalar.activation(out=qt[:, mt, :], in_=ps, func=AF.Copy, scale=0.125)

                # v [P, KT, H, DH] normal-layout values
                wv = pb2.tile([P, DT, D], F32R, tag="w8", name=f"{pfx}_wv_sb")
                nc.sync.dma_start(out=wv, in_=Wd[pfx + "wv"].rearrange("(k p) o -> p k o", p=P))
                v_sb = pb1.tile([P, KT, H, DH], F32R, tag="v", name=f"{pfx}_v")
                for tkt in range(KT):
                    ps = prot.tile([P, M], F32, tag="rot")
                    for kt in range(DT):
                        nc.tensor.matmul(ps[:], wfT[:, kt, tkt * P:(tkt + 1) * P], wv[:, kt, :],
                                         start=(kt == 0), stop=(kt == DT - 1))
                    nc.vector.tensor_copy(out=v_sb[:, tkt, :, :].rearrange("p h d -> p (h d)"),
                                          in_=ps)

                # attention, head pairs
                wk = pb2.tile([P, DT, D], F32R, tag="w8", name=f"{pfx}_wk_sb")
                nc.sync.dma_start(out=wk, in_=Wd[pfx + "wk"].rearrange("(k p) o -> p k o", p=P))
                oT = pb1.tile([P, DT, M], F32R, tag="oT", name=f"{pfx}_oT")
                for hp in range(DT):
                    ktile = pb2.tile([P, 2, M], F32R, tag="ktile", name=f"{pfx}_kt{hp}")
                    pss = [prot.tile([P, M], F32, tag="rot", name=f"{pfx}_kps{hp}_{i}")
                           for i in range(2)]
                    for kt in range(DT):
                        src = ksrc_fn(kt, hp)
                        for nch in range(2):
                            nc.tensor.matmul(pss[nch][:], wk[:, kt, hp * P:(hp + 1) * P],
                                             src[:, nch, :],
                                             start=(kt == 0), stop=(kt == DT - 1))
                    for nch in range(2):
                        nc.scalar.activation(out=ktile[:, nch, :], in_=pss[nch], func=AF.Copy)
                    kflat = ktile.rearrange("p c m -> p (c m)")
                    for h01 in range(2):
                        h = 2 * hp + h01
                        hs = 64 * h01
                        ch = float(CH[h])
                        ops = pacc.tile([P, M], F32, tag="ops", name=f"{pfx}_ops{h}")
                        se = prow.tile([1, M], F32, tag="row", name=f"{pfx}_se{h}")
                        for kt in range(KT):
                            bt = pb3.tile([P, M], F32R, tag="bt", name=f"{pfx}_bt{h}_{kt}")
                            if h01 == 0:
                                # Relu(ch*d2 - LO) = max(ch*d2, LO) - LO  (shift cancels in softmax)
                                nc.scalar.activation(out=bt, in_=d2[:, kt, :], func=AF.Relu,
                                                     scale=ch, bias=neglo_t[:, 0:1])
                            else:
                                nc.vector.tensor_scalar(out=bt, in0=d2[:, kt, :], scalar1=ch,
                                                        scalar2=LO, op0=OP.mult, op1=OP.max)
                            lp = prot.tile([P, M], F32, tag="rot", name=f"{pfx}_lp{h}_{kt}")
                            nc.tensor.matmul(lp[:], kflat[hs:hs + 64, kt * P:(kt + 1) * P],
                                             qt[hs:hs + 64, hp, :], start=True, stop=False)
                            nc.tensor.matmul(lp[:], ident, bt, start=False, stop=True,
                                             skip_group_check=True)
                            ex = pb3.tile([P, M], F32R, tag="ex", name=f"{pfx}_ex{h}_{kt}")
                            nc.scalar.activation(out=ex, in_=lp, func=AF.Exp,
                                                 bias=pad_sb[:, kt:kt + 1])
                            nc.tensor.matmul(ops[0:64, :], v_sb[:, kt, h, :], ex,
                                             start=(kt == 0), stop=(kt == KT - 1),
                                             skip_group_check=True)
                            nc.tensor.matmul(se[0:1, :], ones_col[:, 0:1], ex,
                                             start=(kt == 0), stop=(kt == KT - 1),
                                             skip_group_check=True)
                        rr = pb2.tile([1, M], F32R, tag="rr", name=f"{pfx}_rr{h}")
                        with nc.allow_low_precision(reason="f32r rounding of softmax recip"):
                            nc.vector.reciprocal(out=rr, in_=se[0:1, :])
                        bp = prot.tile([P, M], F32, tag="rot", name=f"{pfx}_bp{h}")
                        nc.tensor.matmul(bp[0:64, :], ones_row[0:1, 0:64], rr, start=True, stop=True)
                        bs = pb2.tile([64, M], F32, tag="bs", name=f"{pfx}_bs{h}")
                        nc.scalar.activation(out=bs, in_=bp[0:64, :], func=AF.Copy)
                        if h01 == 0:
                            nc.vector.tensor_tensor(out=oT[0:64, hp, :], in0=ops[0:64, :],
                                                    in1=bs, op=OP.mult)
                        else:
                            ott = pb2.tile([64, M], F32R, tag="oth", name=f"{pfx}_ott{h}")
                            nc.vector.tensor_tensor(out=ott, in0=ops[0:64, :], in1=bs, op=OP.mult)
                            nc.sync.dma_start(out=oT[64:128, hp, :], in_=ott)

                # output projection + residual -> x1
                wo = pb2.tile([P, DT, D], F32R, tag="w8", name=f"{pfx}_wo_sb")
                nc.sync.dma_start(out=wo, in_=Wd[pfx + "wo"].rearrange("(k p) o -> p k o", p=P))
                x1 = pb2.tile([P, DT, M], F32R, tag="xbig", name=f"{pfx}_x1")
                for mt in range(DT):
                    ps2 = prot.tile([P, M], F32, tag="rot", name=f"{pfx}_ops2_{mt}")
                    for kt in range(DT):
                        nc.tensor.matmul(ps2[:], wo[:, kt, mt * P:(mt + 1) * P], oT[:, kt, :],
                                         start=(kt == 0), stop=(kt == DT - 1))
                    nc.vector.tensor_tensor(out=x1[:, mt, :], in0=ps2, in1=resid[:, mt, :],
                                            op=OP.add)

                def layer_norm(xin, nm):
                    mps = prow.tile([1, M], F32, tag="row", name=f"{nm}_mps")
                    for kt in range(DT):
                        nc.tensor.matmul(mps[0:1, :], ones_col[:, 0:1], xin[:, kt, :],
                                         start=(kt == 0), stop=(kt == DT - 1))
                    mu_row = pln.tile([1, M], F32R, tag="lnrow", name=f"{nm}_mu")
                    nc.scalar.activation(out=mu_row, in_=mps[0:1, :], func=AF.Copy, scale=1.0 / D)
                    qps = prow.tile([1, M], F32, tag="row", name=f"{nm}_qps")
                    for kt in range(DT):
                        sqt = pb3.tile([P, M], F32R, tag="scr", name=f"{nm}_sq{kt}")
                        nc.vector.tensor_mul(out=sqt, in0=xin[:, kt, :], in1=xin[:, kt, :])
                        nc.tensor.matmul(qps[0:1, :], ones_col[:, 0:1], sqt,
                                         start=(kt == 0), stop=(kt == DT - 1))
                    ex2_row = pln.tile([1, M], F32, tag="lnrow", name=f"{nm}_ex2")
                    nc.scalar.activation(out=ex2_row, in_=qps[0:1, :], func=AF.Copy, scale=1.0 / D)
                    mu2 = pln.tile([1, M], F32, tag="lnrow", name=f"{nm}_mu2")
                    nc.vector.tensor_mul(out=mu2, in0=mu_row, in1=mu_row)
                    var_row = pln.tile([1, M], F32, tag="lnrow", name=f"{nm}_var")
                    nc.vector.tensor_sub(out=var_row, in0=ex2_row, in1=mu2)
                    std_row = pln.tile([1, M], F32, tag="lnrow", name=f"{nm}_std")
                    nc.scalar.activation(out=std_row, in_=var_row, func=AF.Sqrt,
                                         bias=eps_t[0:1, 0:1])
                    rstd_row = pln.tile([1, M], F32R, tag="lnrow", name=f"{nm}_rstd")
                    with nc.allow_low_precision(reason="f32r rounding of LN rstd"):
                        nc.vector.reciprocal(out=rstd_row, in_=std_row)
                    mubc = prot.tile([P, M], F32, tag="rot", name=f"{nm}_mubc")
                    nc.tensor.matmul(mubc[:], ones_row[0:1, :], mu_row, start=True, stop=True)
                    rbc = prot.tile([P, M], F32, tag="rot", name=f"{nm}_rbc")
                    nc.tensor.matmul(rbc[:], ones_row[0:1, :], rstd_row, start=True, stop=True)
                    xout = pb2.tile([P, DT, M], F32R, tag="xbig", name=f"{nm}_xout")
                    for kt in range(DT):
                        tmp = pb3.tile([P, M], F32, tag="scr", name=f"{nm}_tmp{kt}")
                        nc.vector.tensor_tensor(out=tmp, in0=xin[:, kt, :], in1=mubc,
                                                op=OP.subtract)
                        nc.vector.tensor_tensor(out=xout[:, kt, :], in0=tmp, in1=rbc, op=OP.mult)
                    return xout

                x1n = layer_norm(x1, f"{pfx}_ln1")

                # MLP (w2 streamed per hidden tile)
                w1 = pb1.tile([P, DT, DFF], F32R, tag="w1", name=f"{pfx}_w1_sb")
                nc.sync.dma_start(out=w1, in_=Wd[pfx + "w1"].rearrange("(k p) o -> p k o", p=P))
                h2ps = []
                for mt2 in range(DT):
                    t = pacc4.tile([P, M], F32, tag="acc4", name=f"{pfx}_h2ps{mt2}")
                    h2ps.append(t)
                w2r = Wd[pfx + "w2"].rearrange("(k p) o -> p k o", p=P)
                for mt1 in range(FT):
                    w2s = pb2.tile([P, 1, D], F32R, tag="w2s", name=f"{pfx}_w2s{mt1}")
                    nc.sync.dma_start(out=w2s, in_=w2r[:, mt1:mt1 + 1, :])
                    ps = prot.tile([P, M], F32, tag="rot", name=f"{pfx}_h1ps{mt1}")
                    for kt in range(DT):
                        nc.tensor.matmul(ps[:], w1[:, kt, mt1 * P:(mt1 + 1) * P], x1n[:, kt, :],
                                         start=(kt == 0), stop=(kt == DT - 1))
                    h1t = pb3.tile([P, M], F32R, tag="scr", name=f"{pfx}_h1t{mt1}")
                    nc.scalar.activation(out=h1t, in_=ps, func=AF.Gelu_apprx_tanh)
                    for mt2 in range(DT):
                        nc.tensor.matmul(h2ps[mt2][:], w2s[:, 0, mt2 * P:(mt2 + 1) * P], h1t,
                                         start=(mt1 == 0), stop=(mt1 == FT - 1))
                x2 = pb2.tile([P, DT, M], F32R, tag="xbig", name=f"{pfx}_x2")
                for mt in range(DT):
                    nc.vector.tensor_tensor(out=x2[:, mt, :], in0=h2ps[mt], in1=x1n[:, mt, :],
                                            op=OP.add)

                xf = layer_norm(x2, f"{pfx}_ln2")
                nc.sync.dma_start(out=out_d.rearrange("(k p) m -> p k m", p=P), in_=xf)
                return xf

            # ---- phase A: seq branch (q=aas, k=wf, v=wf) ----
            def ksrc_A(kt, hp):
                return wfT[:, kt, :].rearrange("p (c m) -> p c m", c=2)

            aas_new = emit_branch("seq", aasT, ksrc_A, aasT, aas_out)

            # ---- allgather aas_new between core pairs ----
            cin = dram.tile([D, M], F32R, tag="cin")
            cout = dram.tile([N, M], F32R, tag="cout")
            nc.sync.dma_start(out=cin[:].rearrange("(k p) m -> p k m", p=P), in_=aas_new)
            nc.gpsimd.collective_compute(
                "AllGather", OP.bypass,
                replica_groups=[[0, 1], [2, 3], [4, 5], [6, 7]],
                ins=[cin[:]], outs=[cout[:]],
            )
            cout_r = cout[:].rearrange("(c k p) m -> p k c m", c=2, p=P)

            # ---- phase B: struct branch (q=wf_m, k=aas_new_full, v=wf) ----
            def ksrc_B(kt, hp):
                t = pb2.tile([P, 2, M], F32R, tag="ksrcB", name=f"ksb{hp}_{kt}")
                nc.sync.dma_start(out=t, in_=cout_r[:, kt, :, :])
                return t

            emit_branch("str", wfTm, ksrc_B, wfTm, wf_out)

    return nc


def host_prep(wf, aas, coords, key_padding_mask, params):
    wf = np.asarray(wf, np.float32)
    aas = np.asarray(aas, np.float32)
    coords = np.asarray(coords, np.float32)
    mask = np.asarray(key_padding_mask)
    consts = {
        "ident": np.eye(P, dtype=np.float32),
        "ones_col": np.ones((P, 1), np.float32),
        "ones_row": np.ones((1, P), np.float32),
        "ones3": np.ones((3, 1), np.float32),
        "ones_row_f": np.ones((1, P), np.float32),
    }
    wmap = {}
    for pfx, pkey in (("seq", "seq_attn"), ("str", "struct_attn")):
        ap = params[pkey]
        for wn in ("wq", "wk", "wv", "wo"):
            wmap[f"{pfx}_{wn}"] = np.asarray(ap[wn], np.float32)
    for pfx, pkey in (("seq", "seq_ffn"), ("str", "struct_ffn")):
        fp = params[pkey]
        wmap[f"{pfx}_w1"] = np.asarray(fp["w1"], np.float32)
        wmap[f"{pfx}_w2"] = np.asarray(fp["w2"], np.float32)

    in_maps = []
    for c in range(8):
        b, half = c // 2, c % 2
        m0 = half * M
        wfT = np.ascontiguousarray(wf[b].T)
        pad = np.where(mask[b], np.float32(PAD_NEG), np.float32(0.0)).astype(np.float32)
        im = {
            "aasT_s": np.ascontiguousarray(aas[b].T[:, m0:m0 + M]),
            "wfT": wfT,
            "wfT_m": np.ascontiguousarray(wfT[:, m0:m0 + M]),
            "coordsT": np.ascontiguousarray(coords[b].T),
            "coordsT_m": np.ascontiguousarray(coords[b].T[:, m0:m0 + M]),
            "coords_n": coords[b],
            "pad": np.ascontiguousarray(pad.reshape(KT, P).T),
        }
        im.update(consts)
        im.update(wmap)
        in_maps.append(im)
    return in_maps


_CACHE = {}


def _get_nc():
    if "nc" not in _CACHE:
        nc = bacc.Bacc("TRN2", target_bir_lowering=False, debug=False, num_devices=8)
        build(nc)
        nc.compile()
        _CACHE["nc"] = nc
    return _CACHE["nc"]


def kernel(wf, aas, coords, key_padding_mask, params):
    nc = _get_nc()
    in_maps = host_prep(wf, aas, coords, key_padding_mask, params)
    results = bass2jax.run_bass_via_pjrt(nc, in_maps, n_cores=8)
    wf_full = np.zeros((B, N, D), np.float32)
    aas_full = np.zeros((B, N, D), np.float32)
    for c in range(8):
        b, half = c // 2, c % 2
        m0 = half * M
        wf_full[b, m0:m0 + M, :] = results[c]["wf_outT"].T
        aas_full[b, m0:m0 + M, :] = results[c]["aas_outT"].T
    return (wf_full, aas_full)
